# revision 8
# baseline (speedup 1.0000x reference)
"""CubeAttention TRN2 Bass kernel — channel-sharded two-launch design.

Shapes (hardcoded): x (2, 262144, 64) fp32; H=W=D=64, C=64, GROUPS=2.
Core i of 8: batch b=i//4, quarter q=i%4 -> groups g in [8q, 8q+8), channels
c in [16q, 16q+16).

Phase 1 (per core): for each of 8 channel-pair groups: transverse + coronal
convs as 9 accumulated tap-matmuls each (K=128 contraction over (i,h) or
(i,d)), then f = cor^T@tans per w, attn = cor2^T@tans2 per j (after an
on-chip j<->w DMA permute), shortcut multiply, write cat rows (bf16).

Phase 2 (per core): 1x1 map conv over the 128 gathered cat channels
(k-permuted w_map), bias + exact Gelu, write fp32 output slice.
"""
import numpy as np
import ml_dtypes

import concourse.bass as bass
import concourse.bacc as bacc
import concourse.mybir as mybir
import concourse.tile as tile
from concourse.bass_utils import run_bass_kernel_spmd

F32 = mybir.dt.float32
BF16 = mybir.dt.bfloat16
AF = mybir.ActivationFunctionType
ALU = mybir.AluOpType
BF = ml_dtypes.bfloat16

B, C, H, W, D = 2, 64, 64, 64, 64
N = H * W * D
NCHAN = 16          # channels per core
NGRP = 8            # groups per core
TAPS = [(0, 0), (-1, -1), (-1, 0), (-1, 1), (0, -1), (0, 1), (1, -1), (1, 0), (1, 1)]

_cache = {}


def build_phase1():
    nc = bacc.Bacc(None)
    xa_d = nc.dram_tensor("xa", [NGRP, 128, W * D], BF16, kind="ExternalInput")
    xb_d = nc.dram_tensor("xb", [NGRP, 128, H * W], BF16, kind="ExternalInput")
    wt_d = nc.dram_tensor("wt", [9, 128, 128], BF16, kind="ExternalInput")
    wc_d = nc.dram_tensor("wc", [9, 128, 128], BF16, kind="ExternalInput")
    bt_d = nc.dram_tensor("bt", [128, 1], F32, kind="ExternalInput")
    bc_d = nc.dram_tensor("bc", [128, 1], F32, kind="ExternalInput")
    out1_d = nc.dram_tensor("out1", [4 * NGRP, 64, 4096], BF16, kind="ExternalOutput")
    tans_dram = nc.dram_tensor("tans_scratch", [128, 4096], BF16)
    cor_dram = nc.dram_tensor("cor_scratch", [128, 8192], BF16)

    with tile.TileContext(nc) as tc:
        with (
            tc.tile_pool(name="wp", bufs=1) as wp,
            tc.tile_pool(name="xp", bufs=2) as xp,
            tc.tile_pool(name="cp", bufs=1) as cp,
            tc.tile_pool(name="sp", bufs=1) as sp,
            tc.tile_pool(name="pst", bufs=2, space="PSUM") as pst,
            tc.tile_pool(name="psc", bufs=2, space="PSUM") as psc,
            tc.tile_pool(name="psf", bufs=2, space="PSUM") as psf,
            tc.tile_pool(name="psa", bufs=2, space="PSUM") as psa,
        ):
            wt_sb = wp.tile([128, 9 * 128], BF16)
            wc_sb = wp.tile([128, 9 * 128], BF16)
            nc.sync.dma_start(
                wt_sb[:].rearrange("p (t m) -> p t m", t=9), wt_d[:].transpose([1, 0, 2]))
            nc.sync.dma_start(
                wc_sb[:].rearrange("p (t m) -> p t m", t=9), wc_d[:].transpose([1, 0, 2]))
            bt_sb = wp.tile([128, 1], F32)
            bc_sb = wp.tile([128, 1], F32)
            nc.sync.dma_start(bt_sb[:], bt_d[:])
            nc.sync.dma_start(bc_sb[:], bc_d[:])

            tans_sb = cp.tile([128, 4096], BF16)    # (m j), (w d)
            cor_blk = cp.tile([128, 8192], BF16)    # (m j), (w, h_blk128)  m-banded
            tans2 = cp.tile([128, 4096], BF16)      # (m w), (j d)
            cor2_blk = cp.tile([128, 8192], BF16)   # (m w), (j, h_blk128)  m-banded
            sf_st = sp.tile([128, 4096], BF16)
            at_st = sp.tile([128, 4096], BF16)
            nc.vector.memset(cor_blk[:], 0.0)
            nc.vector.memset(cor2_blk[:], 0.0)

            for gl in range(NGRP):
                xa = xp.tile([128, 4096], BF16, tag="xa")   # (m h), (w d)
                xb = xp.tile([128, 4096], BF16, tag="xb")   # (m d), (h w)
                nc.sync.dma_start(xa[:], xa_d[gl])
                nc.sync.dma_start(xb[:], xb_d[gl])
                xa3 = xa[:].rearrange("p (w d) -> p w d", w=64)
                xb3 = xb[:].rearrange("p (h w) -> p h w", h=64)

                # ---- transverse conv: out (w,d), contraction (i,h)+taps ----
                for chk in range(8):
                    w0 = chk * 8
                    pt = pst.tile([128, 512], F32, tag="convt")
                    pt3 = pt[:].rearrange("p (w d) -> p w d", w=8)
                    for ti, (dw, dd) in enumerate(TAPS):
                        aw, bw = max(w0, -dw), min(w0 + 8, 64 - dw)
                        ad, bd = max(0, -dd), min(64, 64 - dd)
                        nc.tensor.matmul(
                            pt3[:, aw - w0:bw - w0, ad:bd],
                            wt_sb[:, ti * 128:(ti + 1) * 128],
                            xa3[:, aw + dw:bw + dw, ad + dd:bd + dd],
                            start=(ti == 0), stop=(ti == 8))
                    nc.vector.tensor_tensor(
                        tans_sb[:, w0 * 64:(w0 + 8) * 64], pt[:],
                        bt_sb[:, 0:1].to_broadcast([128, 512]), op=ALU.add)

                # ---- coronal conv: out (w,h), contraction (i,d)+taps ----
                for chk in range(8):
                    w0 = chk * 8
                    pc = psc.tile([128, 512], F32, tag="convc")
                    pc3 = pc[:].rearrange("p (w h) -> p w h", w=8)
                    for ti, (dh, dw2) in enumerate(TAPS):
                        aw, bw = max(w0, -dw2), min(w0 + 8, 64 - dw2)
                        ah, bh = max(0, -dh), min(64, 64 - dh)
                        nc.tensor.matmul(
                            pc3[:, aw - w0:bw - w0, ah:bh],
                            wc_sb[:, ti * 128:(ti + 1) * 128],
                            xb3[:, ah + dh:bh + dh, aw + dw2:bw + dw2].transpose([0, 2, 1]),
                            start=(ti == 0), stop=(ti == 8))
                    cb3 = cor_blk[:].rearrange("p (w hb) -> p w hb", w=64)
                    for m in range(2):
                        nc.vector.tensor_tensor(
                            cb3[m * 64:(m + 1) * 64, w0:w0 + 8, m * 64:m * 64 + 64],
                            pc3[m * 64:(m + 1) * 64],
                            bc_sb[m * 64:(m + 1) * 64, 0:1].to_broadcast([64, 8, 64]),
                            op=ALU.add)

                # ---- permutes: j<->w on partitions, via DRAM round-trip ----
                nc.sync.dma_start(tans_dram[:], tans_sb[:])
                nc.sync.dma_start(cor_dram[:], cor_blk[:])
                td3 = tans_dram[:].rearrange("p (w d) -> p w d", w=64)
                cd3 = cor_dram[:].rearrange("p (w hb) -> p w hb", w=64)
                t2s = tans2[:].rearrange("p (j d) -> p j d", j=64)
                c2s = cor2_blk[:].rearrange("p (j hb) -> p j hb", j=64)
                for m in range(2):
                    sl = slice(m * 64, m * 64 + 64)
                    hb = slice(m * 64, m * 64 + 64)
                    nc.sync.dma_start(
                        t2s[sl, :, :], td3[sl, :, :].transpose([1, 0, 2]))
                    nc.sync.dma_start(
                        c2s[sl, :, hb], cd3[sl, :, hb].transpose([1, 0, 2]))

                # ---- f + shortcut multiply ----
                for chk in range(8):
                    w0 = chk * 8
                    pf = psf.tile([128, 512], F32, tag="fo")
                    for wl in range(8):
                        w = w0 + wl
                        nc.tensor.matmul(
                            pf[:, wl * 64:(wl + 1) * 64],
                            cor_blk[:, w * 128:(w + 1) * 128],
                            tans_sb[:, w * 64:(w + 1) * 64],
                            start=True, stop=True)
                    nc.vector.tensor_tensor(
                        sf_st[:, w0 * 64:(w0 + 8) * 64], pf[:],
                        xa[:, w0 * 64:(w0 + 8) * 64], op=ALU.mult)

                # ---- attn ----
                for chk in range(8):
                    j0 = chk * 8
                    pa = psa.tile([128, 512], F32, tag="ao")
                    for jl in range(8):
                        j = j0 + jl
                        nc.tensor.matmul(
                            pa[:, jl * 64:(jl + 1) * 64],
                            cor2_blk[:, j * 128:(j + 1) * 128],
                            tans2[:, j * 64:(j + 1) * 64],
                            start=True, stop=True)
                    nc.scalar.activation(
                        at_st[:, j0 * 64:(j0 + 8) * 64], pa[:], AF.Copy)

                # ---- stores ----
                nc.sync.dma_start(out1_d[4 * gl + 0], sf_st[0:64, :])
                nc.sync.dma_start(out1_d[4 * gl + 1], sf_st[64:128, :])
                nc.sync.dma_start(out1_d[4 * gl + 2], at_st[0:64, :])
                nc.sync.dma_start(out1_d[4 * gl + 3], at_st[64:128, :])
    nc.finalize()
    return nc


def build_phase2():
    nc = bacc.Bacc(None)
    cat_d = nc.dram_tensor("cat", [128, N // 4], BF16, kind="ExternalInput")
    wm_d = nc.dram_tensor("wm", [128, 64], BF16, kind="ExternalInput")
    bm_d = nc.dram_tensor("bm", [64, 1], F32, kind="ExternalInput")
    out2_d = nc.dram_tensor("out2", [64, N // 4], F32, kind="ExternalOutput")

    NCH = N // 4 // 2048  # 32 chunks of 2048
    with tile.TileContext(nc) as tc:
        with (
            tc.tile_pool(name="wp", bufs=1) as wp,
            tc.tile_pool(name="cp", bufs=3) as cpool,
            tc.tile_pool(name="sp", bufs=2) as sp,
            tc.tile_pool(name="ps", bufs=4, space="PSUM") as ps,
        ):
            wm_sb = wp.tile([128, 64], BF16)
            bm_sb = wp.tile([64, 1], F32)
            nc.sync.dma_start(wm_sb[:], wm_d[:])
            nc.sync.dma_start(bm_sb[:], bm_d[:])
            for t in range(NCH):
                n0 = t * 2048
                ct = cpool.tile([128, 2048], BF16, tag="ct")
                nc.sync.dma_start(ct[:], cat_d[:, n0:n0 + 2048])
                st = sp.tile([64, 2048], F32, tag="st")
                for s in range(4):
                    pp = ps.tile([64, 512], F32, tag="mm")
                    nc.tensor.matmul(pp[:], wm_sb[:], ct[:, s * 512:(s + 1) * 512],
                                     start=True, stop=True)
                    nc.scalar.activation(st[:, s * 512:(s + 1) * 512], pp[:],
                                         AF.Gelu, bias=bm_sb[:, 0:1])
                nc.sync.dma_start(out2_d[:, n0:n0 + 2048], st[:])
    nc.finalize()
    return nc


def _prep(x, w_t, b_t, w_c, b_c, w_map, b_map):
    xg = np.ascontiguousarray(x.transpose(0, 2, 1)).reshape(B, C, H, W, D)
    xga = xg.astype(BF)
    xgb = np.ascontiguousarray(xg.transpose(0, 1, 4, 2, 3)).astype(BF)
    wt_taps = np.ascontiguousarray(
        w_t.transpose(3, 4, 1, 2, 0)).reshape(9, 128, 128).astype(BF)
    wc_taps = np.ascontiguousarray(
        w_c.transpose(2, 3, 1, 4, 0)).reshape(9, 128, 128).astype(BF)
    # tap order must match TAPS [(0,0), (-1,-1), ...] where index = (dt+1)*3+(dd+1)
    order = [(dw + 1) * 3 + (dd + 1) for (dw, dd) in TAPS]
    wt_taps = wt_taps[order]
    wc_taps = wc_taps[order]
    bt = b_t.reshape(128, 1).astype(np.float32)
    bc = b_c.reshape(128, 1).astype(np.float32)
    # phase-2 k permutation
    pi = np.empty(128, np.int64)
    for p in range(4):
        for gl in range(8):
            for m in range(2):
                pi[32 * p + 4 * gl + m] = 16 * p + 2 * gl + m
                pi[32 * p + 4 * gl + 2 + m] = 64 + 16 * p + 2 * gl + m
    wmT = np.ascontiguousarray(w_map[:, pi].T).astype(BF)
    bm = b_map.reshape(64, 1).astype(np.float32)
    return xg, xga, xgb, wt_taps, wc_taps, bt, bc, wmT, bm


def kernel(x, w_t, b_t, w_c, b_c, w_map, b_map, _bench=None):
    x = np.asarray(x, np.float32)
    xg, xga, xgb, wt_taps, wc_taps, bt, bc, wmT, bm = _prep(
        np.asarray(x, np.float32), np.asarray(w_t, np.float32),
        np.asarray(b_t, np.float32), np.asarray(w_c, np.float32),
        np.asarray(b_c, np.float32), np.asarray(w_map, np.float32),
        np.asarray(b_map, np.float32))

    if "nc1" not in _cache:
        _cache["nc1"] = build_phase1()
        _cache["nc2"] = build_phase2()
    nc1, nc2 = _cache["nc1"], _cache["nc2"]

    in_maps1 = []
    for i in range(8):
        b, q = divmod(i, 4)
        in_maps1.append({
            "xa": np.ascontiguousarray(
                xga[b, 16 * q:16 * q + 16]).reshape(NCHAN, H, W * D),
            "xb": np.ascontiguousarray(
                xgb[b, 16 * q:16 * q + 16]).reshape(NCHAN, D, H * W),
            "wt": wt_taps, "wc": wc_taps, "bt": bt, "bc": bc,
        })
    kw1 = dict(_bench or {})
    r1 = run_bass_kernel_spmd(nc1, in_maps1, core_ids=list(range(8)), **kw1)
    out1 = [r["out1"].reshape(32, N) for r in r1.results]

    in_maps2 = []
    for i in range(8):
        b, q = divmod(i, 4)
        n0 = q * (N // 4)
        cat = np.concatenate(
            [out1[4 * b + p][:, n0:n0 + N // 4] for p in range(4)], axis=0)
        in_maps2.append({"cat": np.ascontiguousarray(cat), "wm": wmT, "bm": bm})
    r2 = run_bass_kernel_spmd(nc2, in_maps2, core_ids=list(range(8)), **kw1)

    out = np.empty((B, N, C), np.float32)
    for i in range(8):
        b, q = divmod(i, 4)
        n0 = q * (N // 4)
        out[b, n0:n0 + N // 4, :] = r2.results[i]["out2"].T
    if _bench is not None:
        return out, (r1, r2)
    return out
